# revision 3
# baseline (speedup 1.0000x reference)
"""Trainium2 Bass kernel for nn_L2neighs_Aggregator (gnn_message_passing).

Data-parallel over the node batch dim N across 8 NeuronCores. Host prepares
feature-major inputs; the device runs the 2-layer MLP, attention MLP,
softmax and attention-weighted reduction with f32r matmuls.
"""
import sys

sys.path.insert(0, "/opt/trn_rl_repo")

import numpy as np

import concourse.bass as bass
import concourse.mybir as mybir
import concourse.tile as tile
from concourse.bass_utils import run_bass_kernel_spmd
from concourse.masks import make_identity

N, K, A = 4096, 64, 8
D = 128
NCORES = 8
NC_N = N // NCORES            # 512 nodes per core
PATHS = NC_N * K              # 32768 paths per core
TP = 512                      # paths per tile
NT = PATHS // TP              # 64 tiles
NODES_PER_TILE = TP // K      # 8

f32 = mybir.dt.float32
f32r = mybir.dt.float32r

_cache = {}


def legalize_waits(nc, max_waits=1):
    """This walrus accepts only one sync-wait per engine instruction; move
    excess waits onto injected per-engine NoOps (one wait each)."""
    n = 0
    for fn in nc.m.functions:
        for bb in fn.blocks:
            out = []
            for inst in bb.instructions:
                si = inst.sync_info
                if si is not None and si.on_wait and len(si.on_wait) > max_waits:
                    extra, keep = si.on_wait[:-max_waits], si.on_wait[-max_waits:]
                    for w in extra:
                        n += 1
                        out.append(
                            mybir.InstNoOp(
                                name=f"waitnop-{n}-{inst.name}",
                                engine=inst.engine,
                                ins=[],
                                outs=[],
                                sync_info=mybir.SyncInfo(on_wait=[w], on_update=[]),
                            )
                        )
                    si.on_wait = keep
                out.append(inst)
            bb.instructions[:] = out
    return n


def build():
    nc = bass.Bass()
    xt = nc.dram_tensor("xt", [4 * D, PATHS], f32r, kind="ExternalInput")
    selfb = nc.dram_tensor("selfb", [D, PATHS], f32r, kind="ExternalInput")
    w1 = nc.dram_tensor("w1", [4 * D, 2 * D], f32r, kind="ExternalInput")
    w2 = nc.dram_tensor("w2", [2 * D, D], f32r, kind="ExternalInput")
    a1 = nc.dram_tensor("a1", [2 * D, D], f32r, kind="ExternalInput")
    a2 = nc.dram_tensor("a2", [D, D], f32r, kind="ExternalInput")
    a3bc = nc.dram_tensor("a3bc", [D, D], f32r, kind="ExternalInput")
    b1t = nc.dram_tensor("b1t", [D, 2], f32, kind="ExternalInput")
    b2t = nc.dram_tensor("b2t", [D, 1], f32, kind="ExternalInput")
    ab1t = nc.dram_tensor("ab1t", [D, 1], f32, kind="ExternalInput")
    ab2t = nc.dram_tensor("ab2t", [D, 1], f32, kind="ExternalInput")
    ones = nc.dram_tensor("ones", [1, D], f32, kind="ExternalInput")
    out = nc.dram_tensor("out", [NC_N, D], f32, kind="ExternalOutput")

    Relu = mybir.ActivationFunctionType.Relu
    Exp = mybir.ActivationFunctionType.Exp
    Copy = mybir.ActivationFunctionType.Copy

    with tile.TileContext(nc) as tc:
        with (
            tc.tile_pool(name="const", bufs=1) as cp,
            tc.tile_pool(name="sb", bufs=3) as sb,
            tc.tile_pool(name="acc", bufs=1) as accp,
            tc.tile_pool(name="ps", bufs=1, space="PSUM") as ps,
        ):
            w1_sb = cp.tile([D, 4, 2 * D], f32r)
            nc.sync.dma_start(w1_sb[:], xt_ap(w1[:], 4, D, 2 * D))
            w2_sb = cp.tile([D, 2, D], f32r)
            nc.sync.dma_start(w2_sb[:], xt_ap(w2[:], 2, D, D))
            a1_sb = cp.tile([D, 2, D], f32r)
            nc.sync.dma_start(a1_sb[:], xt_ap(a1[:], 2, D, D))
            a2_sb = cp.tile([D, D], f32r)
            nc.sync.dma_start(a2_sb[:], a2[:])
            a3_sb = cp.tile([D, D], f32r)
            nc.sync.dma_start(a3_sb[:], a3bc[:])
            b1_sb = cp.tile([D, 2], f32)
            nc.sync.dma_start(b1_sb[:], b1t[:])
            b2_sb = cp.tile([D, 1], f32)
            nc.sync.dma_start(b2_sb[:], b2t[:])
            ab1_sb = cp.tile([D, 1], f32)
            nc.sync.dma_start(ab1_sb[:], ab1t[:])
            ab2_sb = cp.tile([D, 1], f32)
            nc.sync.dma_start(ab2_sb[:], ab2t[:])
            ones_sb = cp.tile([D, D], f32)
            nc.sync.dma_start(ones_sb[:1, :], ones[:])
            ident = cp.tile([D, D], f32)
            make_identity(nc, ident[:])

            outT = accp.tile([D, NC_N], f32)      # [feat, node] accumulator
            sums_t = accp.tile([D, NC_N], f32)
            sums = sums_t[:1, :]                  # per-node sum of exp

            for t in range(NT):
                sl = slice(t * TP, (t + 1) * TP)
                x_sb = sb.tile([D, 4, TP], f32r, tag="x")
                for c in range(4):
                    nc.sync.dma_start(
                        x_sb[:, c, :], xt[c * D:(c + 1) * D, sl]
                    )
                sf_sb = sb.tile([D, TP], f32r, tag="sf")
                nc.sync.dma_start(sf_sb[:], selfb[:, sl])

                h1p = ps.tile([D, 2, TP], f32, tag="h1p")
                for m in range(2):
                    for c in range(4):
                        nc.tensor.matmul(
                            h1p[:, m, :],
                            w1_sb[:, c, m * D:(m + 1) * D],
                            x_sb[:, c, :],
                            start=(c == 0),
                            stop=(c == 3),
                        )
                h1 = sb.tile([D, 2, TP], f32r, tag="h1")
                for m in range(2):
                    nc.scalar.activation(
                        h1[:, m, :], h1p[:, m, :], Relu, bias=b1_sb[:, m:m + 1]
                    )

                h2p = ps.tile([D, TP], f32, tag="h2p")
                for c in range(2):
                    nc.tensor.matmul(
                        h2p[:], w2_sb[:, c, :], h1[:, c, :],
                        start=(c == 0), stop=(c == 1),
                    )
                h2 = sb.tile([D, TP], f32r, tag="h2")
                nc.scalar.activation(h2[:], h2p[:], Relu, bias=b2_sb[:, :1])

                a1p = ps.tile([D, TP], f32, tag="a1p")
                nc.tensor.matmul(a1p[:], a1_sb[:, 0, :], h2[:], start=True, stop=False)
                nc.tensor.matmul(a1p[:], a1_sb[:, 1, :], sf_sb[:], start=False, stop=True)
                a1v = sb.tile([D, TP], f32r, tag="a1v")
                nc.scalar.activation(a1v[:], a1p[:], Relu, bias=ab1_sb[:, :1])

                a2p = ps.tile([D, TP], f32, tag="a2p")
                nc.tensor.matmul(a2p[:], a2_sb[:], a1v[:], start=True, stop=True)
                a2v = sb.tile([D, TP], f32r, tag="a2v")
                nc.scalar.activation(a2v[:], a2p[:], Relu, bias=ab2_sb[:, :1])

                # logits broadcast across partitions: every column of a3bc is A3
                lp = ps.tile([D, TP], f32, tag="lp")
                nc.tensor.matmul(lp[:], a3_sb[:], a2v[:], start=True, stop=True)
                ebc = sb.tile([D, TP], f32, tag="ebc")
                nc.scalar.activation(ebc[:], lp[:], Exp)

                hw = sb.tile([D, TP], f32, tag="hw")
                nc.vector.tensor_mul(hw[:], h2[:].bitcast(f32), ebc[:])
                nsl = slice(t * NODES_PER_TILE, (t + 1) * NODES_PER_TILE)
                nc.vector.tensor_reduce(
                    outT[:, nsl],
                    hw[:].rearrange("p (n k) -> p n k", k=K),
                    axis=mybir.AxisListType.X,
                    op=mybir.AluOpType.add,
                )
                nc.vector.tensor_reduce(
                    sums[:, nsl],
                    ebc[:1, :].rearrange("p (n k) -> p n k", k=K),
                    axis=mybir.AxisListType.X,
                    op=mybir.AluOpType.add,
                )

            # normalize: out[:, n] /= sums[n], then transpose out to [node, feat]
            rec_t = accp.tile([D, NC_N], f32)
            rec = rec_t[:1, :]
            nc.vector.reciprocal(rec, sums)
            rbc = ps.tile([D, NC_N], f32, tag="rbc")
            nc.tensor.matmul(rbc[:], ones_sb[:1, :], rec, start=True, stop=True)
            onorm = accp.tile([D, NC_N], f32)
            nc.vector.tensor_mul(onorm[:], outT[:], rbc[:])
            for c in range(NC_N // D):
                trp = ps.tile([D, D], f32, tag="trp")
                nc.tensor.transpose(
                    trp[:], onorm[:, c * D:(c + 1) * D], ident[:]
                )
                trs = sb.tile([D, D], f32, tag="trs")
                nc.scalar.activation(trs[:], trp[:], Copy)
                nc.sync.dma_start(out[c * D:(c + 1) * D, :], trs[:])

    legalize_waits(nc)
    return nc


def xt_ap(ap, c, p, n):
    return ap.rearrange("(c p) n -> p c n", p=p)


def kernel(nodes, paths_rel, paths_nbr, attrs, u2e, r2e, ua2e,
           W1, b1, W2, b2, A1, ab1, A2, ab2, A3, ab3):
    nodes = np.asarray(nodes)
    paths_rel = np.asarray(paths_rel)
    paths_nbr = np.asarray(paths_nbr)
    attrs = np.asarray(attrs)
    u2e = np.asarray(u2e, dtype=np.float32)
    r2e = np.asarray(r2e, dtype=np.float32)
    ua2e = np.asarray(ua2e, dtype=np.float32)
    W1 = np.asarray(W1, dtype=np.float32)
    b1 = np.asarray(b1, dtype=np.float32)
    W2 = np.asarray(W2, dtype=np.float32)
    b2 = np.asarray(b2, dtype=np.float32)
    A1 = np.asarray(A1, dtype=np.float32)
    ab1 = np.asarray(ab1, dtype=np.float32)
    A2 = np.asarray(A2, dtype=np.float32)
    ab2 = np.asarray(ab2, dtype=np.float32)
    A3 = np.asarray(A3, dtype=np.float32)

    # host gather + feature-major layout (ab3 cancels in softmax)
    r1 = r2e[paths_rel[..., 0]]
    r2 = r2e[paths_rel[..., 1]]
    ne = u2e[paths_nbr]
    ae = ua2e[attrs].sum(axis=2)
    x = np.concatenate([r1, r2, ne, ae], axis=-1)        # [N, K, 4D]
    xt_full = np.ascontiguousarray(
        x.reshape(N * K, 4 * D).T
    ).astype(np.float32)                                  # [4D, N*K]
    self_e = u2e[nodes]                                   # [N, D]
    selfb_full = np.ascontiguousarray(
        np.repeat(self_e, K, axis=0).T
    ).astype(np.float32)                                  # [D, N*K]

    if "nc" not in _cache:
        _cache["nc"] = build()
    nc = _cache["nc"]

    common = dict(
        w1=W1, w2=W2, a1=A1, a2=A2,
        a3bc=np.ascontiguousarray(np.tile(A3, (1, D))).astype(np.float32),
        b1t=np.ascontiguousarray(b1.reshape(2, D).T),
        b2t=b2.reshape(D, 1),
        ab1t=ab1.reshape(D, 1),
        ab2t=ab2.reshape(D, 1),
        ones=np.ones((1, D), np.float32),
    )
    in_maps = []
    for c in range(NCORES):
        sl = slice(c * PATHS, (c + 1) * PATHS)
        m = dict(common)
        m["xt"] = np.ascontiguousarray(xt_full[:, sl])
        m["selfb"] = np.ascontiguousarray(selfb_full[:, sl])
        in_maps.append(m)

    res = run_bass_kernel_spmd(nc, in_maps, core_ids=list(range(NCORES)))
    outs = [res.results[c]["out"] for c in range(NCORES)]
    return np.concatenate(outs, axis=0).astype(np.float32)
